# revision 11
# baseline (speedup 1.0000x reference)
"""Trainium2 Bass kernel for Dynamic ReLU-B (nn_Dynamic_Relu_B_70291434766473).

Reference computation (per sample n, channel c, pixel p):
    pooled[n,c] = mean_p x[n,c,p]
    h = relu(pooled @ fc1_w.T + fc1_b)                       # [N, 32]
    delta = 2*sigmoid(einsum('koh,nh->kno', fc2_w, h) + fc2_b) - 1
    alpha = delta[..., 0::2]; beta = delta[..., 1::2]        # [K, N, C]
    a = [1,0][k] + 1.0*alpha ; b = [1,0][k] + 0.5*beta
    out = max_k (x * a[k] + b[k])

Strategy: pure data parallel over batch N=32 across 8 NeuronCores (4
samples/core), with the whole streaming pipeline in bf16:

  - the host pre-casts x to bf16 and the device stores bf16 outputs the
    host upcasts, halving HBM traffic in both directions (12.8 MB/core
    total). The harness error gate is 2e-2; bf16 keeps us ~5e-3.
  - bf16 also unlocks the DVE high-throughput modes: tensor_scalar
    (y = x*a+b, per-partition scalars stay fp32) runs 4x, the branch
    max (tensor_tensor) runs 2x.
  - pooling via ScalarE activation(Copy, accum_out) over the bf16
    tiles (1/HW folded into fc1 weights host-side); sample 0 loads and
    pools in pixel-quarters (ch1 on the otherwise-idle DVE) with the
    partials summed by tiny DVE reduces, so the MLP pipeline starts
    during the DMA ramp.
  - MLP per sample with the fc2 weight chunks [33, 128] in bf16 as the
    *stationary* matmul operand: the [128, 8] result lands directly on
    channel partitions, so no PE transpose pass and the tanh
    (2*sigmoid(z)-1 = tanh(z/2)) shrinks to 8 elements/partition.
  - ACT and DVE instructions carry explicit emission-order edges:
    Tile's list scheduler otherwise reorders the ACT queue and pushes
    the head sample's MLP ops behind later pools, delaying the whole
    DVE apply stream.
  - x loads and out stores ride the two HWDGE rings (ch0 -> nc.sync,
    ch1 -> nc.scalar); constants ride the SWDGE (gpsimd) queues.
"""

import numpy as np

N, C, H, W = 32, 256, 56, 56
HW = H * W
HID = C // 8  # 32
NCORES = 8
NPC = N // NCORES  # samples per core

_CACHE = {}


def _build_program():
    """Build (and cache) the compiled Bass program for one core."""
    if "nc" in _CACHE:
        return _CACHE["nc"]

    import concourse.bacc as bacc
    import concourse.mybir as mybir
    import concourse.tile as tile

    f32 = mybir.dt.float32
    bf16 = mybir.dt.bfloat16
    AF = mybir.ActivationFunctionType
    ALU = mybir.AluOpType

    nc = bacc.Bacc(
        "TRN2",
        target_bir_lowering=False,
        debug=False,
        enable_asserts=False,
        num_devices=NCORES,
    )

    xs = nc.dram_tensor("xs", [NPC, C, HW], bf16, kind="ExternalInput").ap()
    w1t = nc.dram_tensor("w1t", [C, HID], f32, kind="ExternalInput").ap()
    fc1b = nc.dram_tensor("fc1b", [HID, 1], f32, kind="ExternalInput").ap()
    w2r = nc.dram_tensor("w2r", [HID + 1, 8 * 128], bf16, kind="ExternalInput").ap()
    out = nc.dram_tensor("out", [NPC, C, HW], bf16, kind="ExternalOutput").ap()

    ring = {0: nc.sync, 1: nc.scalar}  # per-channel-half HWDGE ring
    QW = HW // 4  # pixel-quarter width for the pipeline-head sample

    # per-engine emission-order chains (see module docstring)
    act_chain, dve_chain = [], []

    def A(inst):
        act_chain.append(inst)
        return inst

    def V(inst):
        dve_chain.append(inst)
        return inst

    # sample 3's y1 ops run on ACT to shorten the DVE tail
    ACT_Y1 = {(2, 1), (3, 0), (3, 1)}

    with tile.TileContext(nc) as tc:
        with (
            tc.tile_pool(name="const", bufs=1) as cpool,
            tc.tile_pool(name="x", bufs=2 * NPC) as xpool,
            tc.tile_pool(name="y", bufs=6) as ypool,
            tc.tile_pool(name="o", bufs=6) as opool,
            tc.tile_pool(name="small", bufs=1) as smpool,
            tc.tile_pool(name="ps", bufs=2, space="PSUM") as pspool,
        ):
            # --- constants (SWDGE queues; don't block the HWDGE rings) ---
            w1t_t = []
            for ch in range(2):
                t = cpool.tile([128, HID], f32, tag=f"w1t{ch}")
                nc.gpsimd.dma_start(t[:], w1t[ch * 128:(ch + 1) * 128, :])
                w1t_t.append(t)
            fc1b_t = cpool.tile([HID, 1], f32, tag="fc1b")
            nc.gpsimd.dma_start(fc1b_t[:], fc1b[:])
            w2r_t = cpool.tile([HID + 1, 8 * 128], bf16, tag="w2r")
            nc.gpsimd.dma_start(w2r_t[:], w2r[:])

            # h vectors for all samples; row HID is the fc2-bias ones row
            ht = smpool.tile([HID + 1, NPC], bf16, tag="ht")
            nc.gpsimd.memset(ht[HID:HID + 1, :], 1.0)

            # per-channel trash tiles for the pool dump outputs
            trash = [
                smpool.tile([128, HW], bf16, tag=f"trash{ch}", name=f"trash{ch}")
                for ch in range(2)
            ]

            # --- x loads: ch0 -> sync ring, ch1 -> scalar ring; sample 0
            # streams in pixel-quarters so pooling starts during the ramp ---
            xt = {}
            for n in range(NPC):
                for ch in range(2):
                    t = xpool.tile([128, HW], bf16, tag="x")
                    if n == 0:
                        # pixel-quarters, split over THREE queues (both
                        # HWDGE rings + the SWDGE queue): the DMA ramp is
                        # slow per-queue, so spreading the head sample
                        # over more queues lands it several us earlier
                        for q in range(4):
                            eng = ring[ch] if q < 2 else nc.gpsimd
                            eng.dma_start(
                                t[:, q * QW:(q + 1) * QW],
                                xs[n, ch * 128:(ch + 1) * 128,
                                   q * QW:(q + 1) * QW],
                            )
                    else:
                        ring[ch].dma_start(
                            t[:], xs[n, ch * 128:(ch + 1) * 128, :]
                        )
                    xt[(n, ch)] = t

            pl = {}

            def pool_head_sample(n):
                # pool each pixel-quarter as it lands (ch0 on ACT, ch1 on
                # the idle DVE), accumulating partials in a [128, 4] tile
                # folded by one tiny DVE reduce per channel.
                for ch in range(2):
                    pls = smpool.tile([128, 4], f32, tag=f"pls{n}{ch}")
                    for q in range(4):
                        sl = slice(q * QW, (q + 1) * QW)
                        if ch == 1:
                            V(nc.vector.tensor_scalar(
                                trash[ch][:, sl], xt[(n, ch)][:, sl], 1.0,
                                None, ALU.mult, ALU.add,
                                accum_out=pls[:, q:q + 1],
                            ))
                        else:
                            A(nc.scalar.activation(
                                trash[ch][:, sl], xt[(n, ch)][:, sl],
                                AF.Copy, accum_out=pls[:, q:q + 1],
                            ))
                    pl[(n, ch)] = pls

            def reduce_head_sample(n):
                # fold the [128, 4] partials; ch1 first (its data chain
                # finishes first), both on DVE
                for ch in (1, 0):
                    pls = pl[(n, ch)]
                    p = smpool.tile([128, 1], f32, tag=f"pl{n}{ch}")
                    sc4 = smpool.tile([128, 4], f32, tag=f"sc4{n}{ch}")
                    V(nc.vector.tensor_scalar(
                        sc4[:], pls[:], 1.0, None, ALU.mult, ALU.add,
                        accum_out=p[:],
                    ))
                    pl[(n, ch)] = p

            def pool_sample(n):
                # accum_out = sum over pixels -> [128, 1] fp32; the dump
                # output goes to the per-channel trash tile.
                for ch in range(2):
                    p = smpool.tile([128, 1], f32, tag=f"pl{n}{ch}")
                    A(nc.scalar.activation(
                        trash[ch][:], xt[(n, ch)][:],
                        AF.Copy, accum_out=p[:],
                    ))
                    pl[(n, ch)] = p

            tts = {}

            def mlp_sample(s):
                # fc1: ph = (fc1_w/HW) @ xsum; ch1 term first (its pooled
                # value is ready first for the head sample)
                ph = pspool.tile([HID, 1], f32, tag="ph")
                for ti, ch in enumerate((1, 0)):
                    nc.tensor.matmul(
                        ph[:], w1t_t[ch][:], pl[(s, ch)][:],
                        start=(ti == 0), stop=(ti == 1),
                    )
                A(nc.scalar.activation(
                    ht[0:HID, s:s + 1], ph[:],
                    AF.Relu, bias=fc1b_t[:], scale=1.0,
                ))
                # fc2 with the bf16 [33, 128] weight chunks stationary: the
                # result [128, 8] lands channels-on-partitions directly.
                # col j = k*4 + isbeta*2 + ch (see make_inputs).
                pz = pspool.tile([128, 8], f32, tag="pz")
                for j in range(8):
                    nc.tensor.matmul(
                        pz[:, j:j + 1],
                        w2r_t[:, j * 128:(j + 1) * 128], ht[:, s:s + 1],
                        start=True, stop=True,
                    )
                # t = tanh((z+b2)/2) = 2*sigmoid(z+b2) - 1
                tt = smpool.tile([128, 8], f32, tag=f"tt{s}")
                A(nc.scalar.activation(tt[:], pz[:], AF.Tanh, bias=0.0, scale=0.5))
                tts[s] = tt

            def apply_sample(s, sls=(slice(0, HW),)):
                # y0 = x*a0+b0 and y1 = x*a1+b1 run 4x on DVE (bf16),
                # the branch max runs 2x; store per channel half.
                # the tiny a/b extractions run on DVE right before the
                # stream:  a0 = 1 + tt[:,0:2]   b0 = 1 + 0.5*tt[:,2:4]
                #          a1 = tt[:,4:6]       b1 = 0.5*tt[:,6:8]
                tt = tts[s]
                ab = smpool.tile([128, 8], f32, tag=f"ab{s}")
                V(nc.vector.tensor_scalar_add(ab[:, 0:2], tt[:, 0:2], 1.0))
                V(nc.vector.tensor_scalar(
                    ab[:, 2:4], tt[:, 2:4], 0.5, 1.0, ALU.mult, ALU.add
                ))
                V(nc.vector.tensor_scalar_mul(ab[:, 6:8], tt[:, 6:8], 0.5))
                for sl in sls:
                    w = sl.stop - sl.start
                    for ch in range(2):
                        y0 = ypool.tile([128, w], bf16, tag="y")
                        y1 = ypool.tile([128, w], bf16, tag="y")
                        V(nc.vector.tensor_scalar(
                            y0[:], xt[(s, ch)][:, sl],
                            ab[:, ch:ch + 1], ab[:, 2 + ch:3 + ch],
                            ALU.mult, ALU.add,
                        ))
                        if (s, ch) in ACT_Y1:
                            A(nc.scalar.activation(
                                y1[:], xt[(s, ch)][:, sl], AF.Identity,
                                bias=ab[:, 6 + ch:7 + ch],
                                scale=tt[:, 4 + ch:5 + ch],
                            ))
                        else:
                            V(nc.vector.tensor_scalar(
                                y1[:], xt[(s, ch)][:, sl],
                                tt[:, 4 + ch:5 + ch], ab[:, 6 + ch:7 + ch],
                                ALU.mult, ALU.add,
                            ))
                        o = opool.tile([128, w], bf16, tag="o")
                        V(nc.vector.tensor_max(o[:], y0[:], y1[:]))
                        # stores dispatch from compute-free queues so the
                        # wait-on-max can't head-of-line block compute:
                        # ch0 via the Sync HWDGE ring, ch1 via SWDGE
                        seng = nc.sync if ch == 0 else nc.gpsimd
                        seng.dma_start(
                            out[s, ch * 128:(ch + 1) * 128, sl], o[:]
                        )

            pool_head_sample(0)
            reduce_head_sample(0)
            mlp_sample(0)
            pool_sample(1)
            mlp_sample(1)
            apply_sample(0)
            pool_sample(2)
            mlp_sample(2)
            apply_sample(1)
            pool_sample(3)
            mlp_sample(3)
            apply_sample(2)
            apply_sample(3, sls=(slice(0, HW // 2), slice(HW // 2, HW)))

            # lock ACT and DVE queue order to emission order
            for chain in (act_chain, dve_chain):
                for prev, nxt in zip(chain[:-1], chain[1:]):
                    tile.add_dep_helper(
                        nxt.ins, prev.ins, sync=False, reason="emission order"
                    )

    nc.compile()
    _CACHE["nc"] = nc
    return nc


def make_inputs(x, fc1_w, fc1_b, fc2_w, fc2_b):
    """Host-side prep: shard + bf16-cast x, rearrange weights."""
    import ml_dtypes

    bf16 = ml_dtypes.bfloat16
    x = np.ascontiguousarray(x, dtype=np.float32).reshape(N, C, HW).astype(bf16)
    # fc1: transpose + fold the 1/HW pooling normalizer into the weights
    w1t = np.ascontiguousarray(fc1_w.T.astype(np.float32) / np.float32(HW))
    fc1b = np.ascontiguousarray(fc1_b.astype(np.float32).reshape(HID, 1))
    # fc2 stationary chunks: [HID+1, 1024] with col o=j*128+c,
    # j = k*4 + isbeta*2 + ch; row HID carries fc2_b (ones-row trick)
    w2r = np.zeros((HID + 1, 8 * 128), np.float32)
    for k in range(2):
        for isbeta in range(2):
            wab = fc2_w[k, isbeta::2, :].astype(np.float32)  # [256, 32]
            bab = fc2_b[k, isbeta::2].astype(np.float32)     # [256]
            for ch in range(2):
                j = k * 4 + isbeta * 2 + ch
                sl = slice(j * 128, (j + 1) * 128)
                w2r[:HID, sl] = wab[128 * ch:128 * (ch + 1), :].T
                w2r[HID, sl] = bab[128 * ch:128 * (ch + 1)]
    w2r = w2r.astype(bf16)
    in_maps = []
    for i in range(NCORES):
        in_maps.append({
            "xs": np.ascontiguousarray(x[NPC * i:NPC * (i + 1)]),
            "w1t": w1t,
            "fc1b": fc1b,
            "w2r": w2r,
        })
    return in_maps


def kernel(x, fc1_w, fc1_b, fc2_w, fc2_b):
    from concourse.bass_utils import run_bass_kernel_spmd

    nc = _build_program()
    in_maps = make_inputs(x, fc1_w, fc1_b, fc2_w, fc2_b)
    res = run_bass_kernel_spmd(nc, in_maps, core_ids=list(range(NCORES)))
    shards = [np.asarray(res.results[i]["out"]) for i in range(NCORES)]
    full = np.concatenate(shards, axis=0).astype(np.float32)
    return full.reshape(N, C, H, W)


if __name__ == "__main__":
    rng = np.random.default_rng(0)
    x = rng.standard_normal((N, C, H, W), dtype=np.float32)
    fc1_w = rng.standard_normal((HID, C), dtype=np.float32) * 0.06
    fc1_b = rng.standard_normal((HID,), dtype=np.float32) * 0.06
    fc2_w = rng.standard_normal((2, 2 * C, HID), dtype=np.float32) * 0.17
    fc2_b = rng.standard_normal((2, 2 * C), dtype=np.float32) * 0.17
    out = kernel(x, fc1_w, fc1_b, fc2_w, fc2_b)
    print(out.shape, out.dtype)


# revision 12
# speedup vs baseline: 1.1343x; 1.1343x over previous
"""Trainium2 Bass kernel for Dynamic ReLU-B (nn_Dynamic_Relu_B_70291434766473).

Reference computation (per sample n, channel c, pixel p):
    pooled[n,c] = mean_p x[n,c,p]
    h = relu(pooled @ fc1_w.T + fc1_b)                       # [N, 32]
    delta = 2*sigmoid(einsum('koh,nh->kno', fc2_w, h) + fc2_b) - 1
    alpha = delta[..., 0::2]; beta = delta[..., 1::2]        # [K, N, C]
    a = [1,0][k] + 1.0*alpha ; b = [1,0][k] + 0.5*beta
    out = max_k (x * a[k] + b[k])

Strategy: pure data parallel over batch N=32 across 8 NeuronCores (4
samples/core), with the whole streaming pipeline in bf16:

  - the host pre-casts x to bf16 and the device stores bf16 outputs the
    host upcasts, halving HBM traffic in both directions (12.8 MB/core
    total). The harness error gate is 2e-2; bf16 keeps us ~5e-3.
  - bf16 also unlocks the DVE high-throughput modes: tensor_scalar
    (y = x*a+b, per-partition scalars stay fp32) runs 4x, the branch
    max (tensor_tensor) runs 2x.
  - pooling via ScalarE activation(Copy, accum_out) over the bf16
    tiles (1/HW folded into fc1 weights host-side); sample 0 loads and
    pools in pixel-quarters (ch1 on the otherwise-idle DVE) with the
    partials summed by tiny DVE reduces, so the MLP pipeline starts
    during the DMA ramp.
  - MLP per sample with the fc2 weight chunks [33, 128] in bf16 as the
    *stationary* matmul operand: the [128, 8] result lands directly on
    channel partitions, so no PE transpose pass and the tanh
    (2*sigmoid(z)-1 = tanh(z/2)) shrinks to 8 elements/partition.
  - ACT and DVE instructions carry explicit emission-order edges:
    Tile's list scheduler otherwise reorders the ACT queue and pushes
    the head sample's MLP ops behind later pools, delaying the whole
    DVE apply stream.
  - x loads and out stores ride the two HWDGE rings (ch0 -> nc.sync,
    ch1 -> nc.scalar); constants ride the SWDGE (gpsimd) queues.
"""

import numpy as np

N, C, H, W = 32, 256, 56, 56
HW = H * W
HID = C // 8  # 32
NCORES = 8
NPC = N // NCORES  # samples per core

_CACHE = {}


def _build_program():
    """Build (and cache) the compiled Bass program for one core."""
    if "nc" in _CACHE:
        return _CACHE["nc"]

    import concourse.bacc as bacc
    import concourse.mybir as mybir
    import concourse.tile as tile

    f32 = mybir.dt.float32
    bf16 = mybir.dt.bfloat16
    AF = mybir.ActivationFunctionType
    ALU = mybir.AluOpType

    nc = bacc.Bacc(
        "TRN2",
        target_bir_lowering=False,
        debug=False,
        enable_asserts=False,
        num_devices=NCORES,
    )

    xs = nc.dram_tensor("xs", [NPC, C, HW], bf16, kind="ExternalInput").ap()
    w1t = nc.dram_tensor("w1t", [C, HID], f32, kind="ExternalInput").ap()
    fc1b = nc.dram_tensor("fc1b", [HID, 1], f32, kind="ExternalInput").ap()
    w2r = nc.dram_tensor("w2r", [HID + 1, 8 * 128], bf16, kind="ExternalInput").ap()
    out = nc.dram_tensor("out", [NPC, C, HW], bf16, kind="ExternalOutput").ap()

    ring = {0: nc.sync, 1: nc.scalar}  # per-channel-half HWDGE ring
    QW = HW // 4  # pixel-quarter width for the pipeline-head sample

    # per-engine emission-order chains (see module docstring)
    act_chain, dve_chain = [], []

    def A(inst):
        act_chain.append(inst)
        return inst

    def V(inst):
        dve_chain.append(inst)
        return inst

    # sample 3's y1 ops run on ACT to shorten the DVE tail
    ACT_Y1 = {(2, 1), (3, 0), (3, 1)}

    with tile.TileContext(nc) as tc:
        with (
            tc.tile_pool(name="const", bufs=1) as cpool,
            tc.tile_pool(name="x", bufs=2 * NPC) as xpool,
            tc.tile_pool(name="y", bufs=6) as ypool,
            tc.tile_pool(name="o", bufs=6) as opool,
            tc.tile_pool(name="small", bufs=1) as smpool,
            tc.tile_pool(name="ps", bufs=2, space="PSUM") as pspool,
        ):
            # --- constants (SWDGE queues; don't block the HWDGE rings) ---
            w1t_t = []
            for ch in range(2):
                t = cpool.tile([128, HID], f32, tag=f"w1t{ch}")
                nc.gpsimd.dma_start(t[:], w1t[ch * 128:(ch + 1) * 128, :])
                w1t_t.append(t)
            fc1b_t = cpool.tile([HID, 1], f32, tag="fc1b")
            nc.gpsimd.dma_start(fc1b_t[:], fc1b[:])
            w2r_t = cpool.tile([HID + 1, 8 * 128], bf16, tag="w2r")
            nc.gpsimd.dma_start(w2r_t[:], w2r[:])

            # h vectors for all samples; row HID is the fc2-bias ones row
            ht = smpool.tile([HID + 1, NPC], bf16, tag="ht")
            nc.gpsimd.memset(ht[HID:HID + 1, :], 1.0)

            # per-channel trash tiles for the pool dump outputs
            trash = [
                smpool.tile([128, HW], bf16, tag=f"trash{ch}", name=f"trash{ch}")
                for ch in range(2)
            ]

            # --- x loads: ch0 -> sync ring, ch1 -> scalar ring; sample 0
            # streams in pixel-quarters so pooling starts during the ramp ---
            xt = {}
            for n in range(NPC):
                for ch in range(2):
                    t = xpool.tile([128, HW], bf16, tag="x")
                    if n == 0:
                        # pixel-quarters: each ring is FIFO, so sample 0's
                        # chunks arrive first and pooling tracks the ramp
                        # (the SWDGE queue is useless here - it has a slow
                        # multi-us startup before first data flows)
                        for q in range(4):
                            ring[ch].dma_start(
                                t[:, q * QW:(q + 1) * QW],
                                xs[n, ch * 128:(ch + 1) * 128,
                                   q * QW:(q + 1) * QW],
                            )
                    else:
                        ring[ch].dma_start(
                            t[:], xs[n, ch * 128:(ch + 1) * 128, :]
                        )
                    xt[(n, ch)] = t

            pl = {}

            def pool_head_sample(n):
                # pool each pixel-quarter as it lands (ch0 on ACT, ch1 on
                # the idle DVE), accumulating partials in a [128, 4] tile
                # folded by one tiny DVE reduce per channel.
                for ch in range(2):
                    pls = smpool.tile([128, 4], f32, tag=f"pls{n}{ch}")
                    for q in range(4):
                        sl = slice(q * QW, (q + 1) * QW)
                        if ch == 1:
                            V(nc.vector.tensor_scalar(
                                trash[ch][:, sl], xt[(n, ch)][:, sl], 1.0,
                                None, ALU.mult, ALU.add,
                                accum_out=pls[:, q:q + 1],
                            ))
                        else:
                            A(nc.scalar.activation(
                                trash[ch][:, sl], xt[(n, ch)][:, sl],
                                AF.Copy, accum_out=pls[:, q:q + 1],
                            ))
                    pl[(n, ch)] = pls

            def reduce_head_sample(n):
                # fold the [128, 4] partials; ch1 first (its data chain
                # finishes first), both on DVE
                for ch in (1, 0):
                    pls = pl[(n, ch)]
                    p = smpool.tile([128, 1], f32, tag=f"pl{n}{ch}")
                    sc4 = smpool.tile([128, 4], f32, tag=f"sc4{n}{ch}")
                    V(nc.vector.tensor_scalar(
                        sc4[:], pls[:], 1.0, None, ALU.mult, ALU.add,
                        accum_out=p[:],
                    ))
                    pl[(n, ch)] = p

            def pool_sample(n):
                # accum_out = sum over pixels -> [128, 1] fp32; the dump
                # output goes to the per-channel trash tile.
                for ch in range(2):
                    p = smpool.tile([128, 1], f32, tag=f"pl{n}{ch}")
                    A(nc.scalar.activation(
                        trash[ch][:], xt[(n, ch)][:],
                        AF.Copy, accum_out=p[:],
                    ))
                    pl[(n, ch)] = p

            tts = {}

            def mlp_sample(s):
                # fc1: ph = (fc1_w/HW) @ xsum; ch1 term first (its pooled
                # value is ready first for the head sample)
                ph = pspool.tile([HID, 1], f32, tag="ph")
                for ti, ch in enumerate((1, 0)):
                    nc.tensor.matmul(
                        ph[:], w1t_t[ch][:], pl[(s, ch)][:],
                        start=(ti == 0), stop=(ti == 1),
                    )
                A(nc.scalar.activation(
                    ht[0:HID, s:s + 1], ph[:],
                    AF.Relu, bias=fc1b_t[:], scale=1.0,
                ))
                # fc2 with the bf16 [33, 128] weight chunks stationary: the
                # result [128, 8] lands channels-on-partitions directly.
                # col j = k*4 + isbeta*2 + ch (see make_inputs).
                pz = pspool.tile([128, 8], f32, tag="pz")
                for j in range(8):
                    nc.tensor.matmul(
                        pz[:, j:j + 1],
                        w2r_t[:, j * 128:(j + 1) * 128], ht[:, s:s + 1],
                        start=True, stop=True,
                    )
                # t = tanh((z+b2)/2) = 2*sigmoid(z+b2) - 1
                tt = smpool.tile([128, 8], f32, tag=f"tt{s}")
                A(nc.scalar.activation(tt[:], pz[:], AF.Tanh, bias=0.0, scale=0.5))
                tts[s] = tt

            def apply_sample(s, sls=(slice(0, HW),)):
                # y0 = x*a0+b0 and y1 = x*a1+b1 run 4x on DVE (bf16),
                # the branch max runs 2x; store per channel half.
                # the tiny a/b extractions run on DVE right before the
                # stream:  a0 = 1 + tt[:,0:2]   b0 = 1 + 0.5*tt[:,2:4]
                #          a1 = tt[:,4:6]       b1 = 0.5*tt[:,6:8]
                tt = tts[s]
                ab = smpool.tile([128, 8], f32, tag=f"ab{s}")
                V(nc.vector.tensor_scalar_add(ab[:, 0:2], tt[:, 0:2], 1.0))
                V(nc.vector.tensor_scalar(
                    ab[:, 2:4], tt[:, 2:4], 0.5, 1.0, ALU.mult, ALU.add
                ))
                V(nc.vector.tensor_scalar_mul(ab[:, 6:8], tt[:, 6:8], 0.5))
                for sl in sls:
                    w = sl.stop - sl.start
                    for ch in range(2):
                        y0 = ypool.tile([128, w], bf16, tag="y")
                        y1 = ypool.tile([128, w], bf16, tag="y")
                        V(nc.vector.tensor_scalar(
                            y0[:], xt[(s, ch)][:, sl],
                            ab[:, ch:ch + 1], ab[:, 2 + ch:3 + ch],
                            ALU.mult, ALU.add,
                        ))
                        if (s, ch) in ACT_Y1:
                            A(nc.scalar.activation(
                                y1[:], xt[(s, ch)][:, sl], AF.Identity,
                                bias=ab[:, 6 + ch:7 + ch],
                                scale=tt[:, 4 + ch:5 + ch],
                            ))
                        else:
                            V(nc.vector.tensor_scalar(
                                y1[:], xt[(s, ch)][:, sl],
                                tt[:, 4 + ch:5 + ch], ab[:, 6 + ch:7 + ch],
                                ALU.mult, ALU.add,
                            ))
                        o = opool.tile([128, w], bf16, tag="o")
                        V(nc.vector.tensor_max(o[:], y0[:], y1[:]))
                        # stores dispatch from compute-free queues so the
                        # wait-on-max can't head-of-line block compute:
                        # ch0 via the Sync HWDGE ring, ch1 via SWDGE
                        seng = nc.sync if ch == 0 else nc.gpsimd
                        seng.dma_start(
                            out[s, ch * 128:(ch + 1) * 128, sl], o[:]
                        )

            pool_head_sample(0)
            reduce_head_sample(0)
            mlp_sample(0)
            pool_sample(1)
            mlp_sample(1)
            apply_sample(0)
            pool_sample(2)
            mlp_sample(2)
            apply_sample(1)
            pool_sample(3)
            mlp_sample(3)
            apply_sample(2)
            apply_sample(3, sls=(slice(0, HW // 2), slice(HW // 2, HW)))

            # lock ACT and DVE queue order to emission order
            for chain in (act_chain, dve_chain):
                for prev, nxt in zip(chain[:-1], chain[1:]):
                    tile.add_dep_helper(
                        nxt.ins, prev.ins, sync=False, reason="emission order"
                    )

    nc.compile()
    _CACHE["nc"] = nc
    return nc


def make_inputs(x, fc1_w, fc1_b, fc2_w, fc2_b):
    """Host-side prep: shard + bf16-cast x, rearrange weights."""
    import ml_dtypes

    bf16 = ml_dtypes.bfloat16
    x = np.ascontiguousarray(x, dtype=np.float32).reshape(N, C, HW).astype(bf16)
    # fc1: transpose + fold the 1/HW pooling normalizer into the weights
    w1t = np.ascontiguousarray(fc1_w.T.astype(np.float32) / np.float32(HW))
    fc1b = np.ascontiguousarray(fc1_b.astype(np.float32).reshape(HID, 1))
    # fc2 stationary chunks: [HID+1, 1024] with col o=j*128+c,
    # j = k*4 + isbeta*2 + ch; row HID carries fc2_b (ones-row trick)
    w2r = np.zeros((HID + 1, 8 * 128), np.float32)
    for k in range(2):
        for isbeta in range(2):
            wab = fc2_w[k, isbeta::2, :].astype(np.float32)  # [256, 32]
            bab = fc2_b[k, isbeta::2].astype(np.float32)     # [256]
            for ch in range(2):
                j = k * 4 + isbeta * 2 + ch
                sl = slice(j * 128, (j + 1) * 128)
                w2r[:HID, sl] = wab[128 * ch:128 * (ch + 1), :].T
                w2r[HID, sl] = bab[128 * ch:128 * (ch + 1)]
    w2r = w2r.astype(bf16)
    in_maps = []
    for i in range(NCORES):
        in_maps.append({
            "xs": np.ascontiguousarray(x[NPC * i:NPC * (i + 1)]),
            "w1t": w1t,
            "fc1b": fc1b,
            "w2r": w2r,
        })
    return in_maps


def kernel(x, fc1_w, fc1_b, fc2_w, fc2_b):
    from concourse.bass_utils import run_bass_kernel_spmd

    nc = _build_program()
    in_maps = make_inputs(x, fc1_w, fc1_b, fc2_w, fc2_b)
    res = run_bass_kernel_spmd(nc, in_maps, core_ids=list(range(NCORES)))
    shards = [np.asarray(res.results[i]["out"]) for i in range(NCORES)]
    full = np.concatenate(shards, axis=0).astype(np.float32)
    return full.reshape(N, C, H, W)


if __name__ == "__main__":
    rng = np.random.default_rng(0)
    x = rng.standard_normal((N, C, H, W), dtype=np.float32)
    fc1_w = rng.standard_normal((HID, C), dtype=np.float32) * 0.06
    fc1_b = rng.standard_normal((HID,), dtype=np.float32) * 0.06
    fc2_w = rng.standard_normal((2, 2 * C, HID), dtype=np.float32) * 0.17
    fc2_b = rng.standard_normal((2, 2 * C), dtype=np.float32) * 0.17
    out = kernel(x, fc1_w, fc1_b, fc2_w, fc2_b)
    print(out.shape, out.dtype)
